# revision 13
# baseline (speedup 1.0000x reference)
"""Multi-head attention (B=4, S=2048, D=512, H=8) on 8 trn2 NeuronCores.

Sharding: core c handles batch b = c//2 and head-group g = c%2 (4 heads,
256 of the 512 model dims). Each core computes its 4 heads' attention and
a partial out-projection [2048, 512]; the host sums the two partials per
batch and adds the output bias.

Device kernel per core (all matmuls bf16 -> f32 PSUM):
  1. QKV projections from pre-transposed xT [512, 2048]:
       Q^T/K^T stored per head, zero-padded from 64 to 128 rows so the
       attention matmuls contract over K=128 (a K=64 matmul leaves half
       the PE array inactive and the HAM clock-gate then never grants
       2.4 GHz; the zero rows are numerically inert).
       V [128, 260] per seq-tile, with a per-head all-ones column
       (injected via the bias) so the P@V matmul also produces softmax
       row-sums. wq/bq are pre-scaled by 1/8 on the host.
  2. Per (q-block, head), flash-style: S^T tile [128, 1024] = K_h^T.Q_h,
     exp on ScalarE (PSUM -> SBUF bf16, double-buffered, software
     pipelined), P^T accumulated into O^T [65, 512] over 16 k-tiles.
     ScalarE (exp) is the saturated engine; the Q/K projection of heads
     2-3 and the out-projection of finished q-blocks are interleaved
     into the PE's slack via an aux work queue.
  3. Normalize: reciprocal of the row-sum row + gpsimd partition
     broadcast + multiply, all off the PE/ACT critical path.
  4. Out-projection per seq-tile: sum_p OT[p](chunk) @ wo[p].
No max-subtraction in softmax: scores are O(1) by construction, exp is
safe, and the reference softmax is shift-invariant.
"""

from collections import deque

import numpy as np
import ml_dtypes

import concourse.bacc as bacc
import concourse.mybir as mybir
from concourse.tile import TileContext
from concourse.bass_utils import run_bass_kernel_spmd

BF16 = mybir.dt.bfloat16
F32 = mybir.dt.float32
AF = mybir.ActivationFunctionType
ALU = mybir.AluOpType

B, S, D = 4, 2048, 512
H_CORE, HD = 4, 64          # heads per core, head dim
DHC = H_CORE * HD           # 256 dims per core
VW = H_CORE * 2 * HD        # 512: V augmented with 64 ones-columns per head
                            # (PV matmul then emits the softmax row-sum
                            # pre-replicated on partitions 64-127, and runs
                            # the full M=128 array)
N_CORES = 8

_CACHE = {}


def build_nc():
    nc = bacc.Bacc("TRN2", target_bir_lowering=False, debug=False,
                   num_devices=N_CORES)

    xT_d = nc.declare_dram_parameter("xT", [D, S], BF16, isOutput=False)
    wq_d = nc.declare_dram_parameter("wq", [D, DHC], BF16, isOutput=False)
    wk_d = nc.declare_dram_parameter("wk", [D, DHC], BF16, isOutput=False)
    wv_d = nc.declare_dram_parameter("wv", [D, VW], BF16, isOutput=False)
    wo_d = nc.declare_dram_parameter("wo", [DHC, D], BF16, isOutput=False)
    bq_d = nc.declare_dram_parameter("bq", [DHC, 1], F32, isOutput=False)
    bk_d = nc.declare_dram_parameter("bk", [DHC, 1], F32, isOutput=False)
    bvb_d = nc.declare_dram_parameter("bvb", [128, VW], F32, isOutput=False)
    out_d = nc.declare_dram_parameter("out", [S, D], F32, isOutput=True)

    NQB = 2          # q blocks of 1024
    QB = 1024
    NKT = S // 128   # 16 k tiles

    with TileContext(nc, num_cores=N_CORES) as tc:
        with (
            tc.tile_pool(name="persist", bufs=1) as pp,
            tc.tile_pool(name="pt_pool", bufs=3) as ptp,
            tc.tile_pool(name="rs_pool", bufs=2) as rsp,
            tc.tile_pool(name="ob_pool", bufs=3) as obp,
        ):
            # preload the exp ACT table before anything else: the first
            # real exp otherwise pays a ~2.7us table load that stalls the
            # whole pipeline
            scr = pp.tile([1, 8], F32, tag="scr", name="scr")
            nc.vector.memset(scr[:], 0.0)
            nc.scalar.activation(scr[:], scr[:], AF.Exp)

            # ---- load inputs (Q/K-proj operands first) ----
            xT = [pp.tile([128, S], BF16, tag=f"xT{i}", name=f"xT{i}") for i in range(4)]
            wq = [pp.tile([128, DHC], BF16, tag=f"wq{i}", name=f"wq{i}") for i in range(4)]
            wk = [pp.tile([128, DHC], BF16, tag=f"wk{i}", name=f"wk{i}") for i in range(4)]
            wv = [pp.tile([128, VW], BF16, tag=f"wv{i}", name=f"wv{i}") for i in range(4)]
            bq = [pp.tile([128, 1], F32, tag=f"bq{p}", name=f"bq{p}") for p in range(2)]
            bk = [pp.tile([128, 1], F32, tag=f"bk{p}", name=f"bk{p}") for p in range(2)]
            for i in range(4):
                nc.sync.dma_start(out=xT[i][:], in_=xT_d[128 * i:128 * (i + 1), :])
                nc.sync.dma_start(out=wq[i][:], in_=wq_d[128 * i:128 * (i + 1), :])
                nc.sync.dma_start(out=wk[i][:], in_=wk_d[128 * i:128 * (i + 1), :])
            for p in range(2):
                nc.sync.dma_start(out=bq[p][:], in_=bq_d[128 * p:128 * (p + 1), :])
                nc.sync.dma_start(out=bk[p][:], in_=bk_d[128 * p:128 * (p + 1), :])
            for i in range(4):
                nc.sync.dma_start(out=wv[i][:], in_=wv_d[128 * i:128 * (i + 1), :])
            bvb = pp.tile([128, VW], F32, tag="bvb")
            nc.sync.dma_start(out=bvb[:], in_=bvb_d[:])
            wo = [pp.tile([128, D], BF16, tag=f"wo{p}", name=f"wo{p}") for p in range(2)]
            for p in range(2):
                nc.sync.dma_start(out=wo[p][:], in_=wo_d[128 * p:128 * (p + 1), :])

            QTh = [pp.tile([128, S], BF16, tag=f"QTh{h}", name=f"QTh{h}")
                   for h in range(H_CORE)]
            KTh = [pp.tile([128, S], BF16, tag=f"KTh{h}", name=f"KTh{h}")
                   for h in range(H_CORE)]
            OT = [pp.tile([128, S], BF16, tag=f"OT{p}", name=f"OT{p}") for p in range(2)]
            V = [pp.tile([128, VW], BF16, tag=f"V{st}", name=f"V{st}") for st in range(NKT)]
            for h in range(H_CORE):
                nc.vector.memset(QTh[h][64:128, :], 0.0)
                nc.vector.memset(KTh[h][64:128, :], 0.0)

            # ---- Q / K projections for heads 0/1 (dout tile 0) ----
            # heads 2/3 (dout tile 1) are deferred into the attention
            # phase as aux work
            with tc.tile_pool(name="qk_ps", bufs=2, space="PSUM") as qkps:
                for (w_sb, b_sb, dst) in ((wq, bq, QTh), (wk, bk, KTh)):
                    ps = qkps.tile([128, S], F32, tag="qk", name="qk")
                    for din in range(4):
                        for st in range(4):
                            nc.tensor.matmul(
                                ps[:, 512 * st:512 * (st + 1)],
                                w_sb[din][:, 0:128],
                                xT[din][:, 512 * st:512 * (st + 1)],
                                start=(din == 0), stop=(din == 3),
                            )
                    for m in range(2):
                        nc.vector.tensor_scalar(
                            out=dst[m][0:64, :],
                            in0=ps[64 * m:64 * (m + 1), :],
                            scalar1=b_sb[0][64 * m:64 * (m + 1), :],
                            scalar2=None, op0=ALU.add,
                        )

            # ---- attention + interleaved aux work ----
            # aux queue: (min_kt, closure) emitted into the PE slack of the
            # ACT-bound attention loop. Carries the V projection (block 0),
            # the Q/K projection of heads 2-3 (block 1), and the finished
            # q-block's out-projection.
            with (
                tc.tile_pool(name="s_ps", bufs=2, space="PSUM") as sps,
                tc.tile_pool(name="o_ps", bufs=3, space="PSUM") as ops,
                tc.tile_pool(name="aux_ps", bufs=1, space="PSUM") as axp,
            ):
                aux = deque()

                def v_proj(st):
                    def run():
                        ps = axp.tile([128, 512], F32, tag="aux", name="aux")
                        for din in range(4):
                            nc.tensor.matmul(
                                ps[:, 0:VW],
                                xT[din][:, 128 * st:128 * (st + 1)],
                                wv[din][:],
                                start=(din == 0), stop=(din == 3),
                            )
                        nc.vector.tensor_tensor(
                            out=V[st][:], in0=ps[:, 0:VW], in1=bvb[:],
                            op=ALU.add)
                    return run

                def qk_dt1_aux(w_sb, b_sb, dst, st):
                    def run():
                        ps = axp.tile([128, 512], F32, tag="aux", name="aux")
                        for din in range(4):
                            nc.tensor.matmul(
                                ps[:],
                                w_sb[din][:, 128:256],
                                xT[din][:, 512 * st:512 * (st + 1)],
                                start=(din == 0), stop=(din == 3),
                            )
                        for m in range(2):
                            nc.vector.tensor_scalar(
                                out=dst[2 + m][0:64, 512 * st:512 * (st + 1)],
                                in0=ps[64 * m:64 * (m + 1), :],
                                scalar1=b_sb[1][64 * m:64 * (m + 1), :],
                                scalar2=None, op0=ALU.add,
                            )
                    return run

                def outproj_aux(st):
                    def run():
                        ps = axp.tile([128, 512], F32, tag="aux", name="aux")
                        for p in range(2):
                            nc.tensor.matmul(
                                ps[:],
                                OT[p][:, 128 * st:128 * (st + 1)],
                                wo[p][:],
                                start=(p == 0), stop=(p == 1),
                            )
                        ob = obp.tile([128, D], F32, tag="ob", name="ob")
                        nc.vector.tensor_copy(ob[:], ps[:])
                        nc.sync.dma_start(
                            out=out_d[128 * st:128 * (st + 1), :], in_=ob[:])
                    return run

                # V[0..1] inline (needed by the first PV steps of block 0);
                # the rest rides the aux queue one step ahead of use
                v_proj(0)()
                v_proj(1)()
                for st in range(2, NKT):
                    aux.append((st - 1, v_proj(st)))
                for st in range(4):
                    aux.append((1, qk_dt1_aux(wq, bq, QTh, st)))
                    aux.append((1, qk_dt1_aux(wk, bk, KTh, st)))

                def block(h, qb):
                    p, m = divmod(h, 2)
                    r0, r1 = 64 * m, 64 * (m + 1)
                    q0 = QB * qb
                    o_acc = []

                    def s_mms(kt):
                        stile = sps.tile([128, QB], F32, tag="s", name="s")
                        for qt in range(2):
                            nc.tensor.matmul(
                                stile[:, 512 * qt:512 * (qt + 1)],
                                KTh[h][:, 128 * kt:128 * (kt + 1)],
                                QTh[h][:, q0 + 512 * qt:q0 + 512 * (qt + 1)],
                                start=True, stop=True,
                            )
                        return stile

                    def exp_pv(kt, stile):
                        pt = ptp.tile([128, QB], BF16, tag="pt", name="pt")
                        nc.scalar.activation(pt[:], stile[:], AF.Exp)
                        for qt in range(2):
                            nc.tensor.matmul(
                                o_acc[qt][:],
                                V[kt][:, 2 * HD * h:2 * HD * (h + 1)],
                                pt[:, 512 * qt:512 * (qt + 1)],
                                start=(kt == 0), stop=(kt == NKT - 1),
                            )

                    prev = s_mms(0)
                    for kt in range(1, NKT):
                        cur = s_mms(kt)
                        if not o_acc:
                            o_acc.extend(
                                ops.tile([128, 512], F32, tag="o",
                                         name="o_acc") for _ in range(2))
                        if aux and kt >= aux[0][0]:
                            aux.popleft()[1]()
                        exp_pv(kt - 1, prev)
                        prev = cur
                    exp_pv(NKT - 1, prev)

                    # free the o_acc PSUM slots with one fast copy each;
                    # rows 64-127 already hold the row-sum replicated, so
                    # normalize = reciprocal + same-base multiply, all DVE
                    osb = []
                    for qt in range(2):
                        t = rsp.tile([128, 512], F32, tag="osb", name="osb")
                        nc.vector.tensor_copy(t[:], o_acc[qt][:])
                        osb.append(t)
                    for qt in range(2):
                        recB = rsp.tile([HD, 512], F32, tag="recB",
                                        name="recB")
                        nc.vector.reciprocal(recB[:], osb[qt][HD:2 * HD, :])
                        nc.vector.tensor_tensor(
                            out=OT[p][r0:r1,
                                      q0 + 512 * qt:q0 + 512 * (qt + 1)],
                            in0=osb[qt][0:HD, :], in1=recB[:],
                            op=ALU.mult,
                        )

                for qb in range(NQB):
                    for h in range(H_CORE):
                        block(h, qb)
                    # out-projection of this q-block rides the aux queue;
                    # min_kt 8+ gives the last head's normalize time to land
                    # (qt0 columns first: their OT multiply lands earlier)
                    for j, st in enumerate(range(8 * qb, 8 * (qb + 1))):
                        aux.append((8 + j, outproj_aux(st)))
                while aux:
                    aux.popleft()[1]()

    nc.compile()
    return nc


def _prep_core(x, wq, bq, wk, bk, wv, bv, wo, bo, b, g):
    hs = slice(DHC * g, DHC * (g + 1))
    xT = np.ascontiguousarray(x[b].T).astype(ml_dtypes.bfloat16)
    wq_c = (wq[:, hs] / 8.0).astype(ml_dtypes.bfloat16)
    bq_c = (bq[hs] / 8.0).astype(np.float32).reshape(DHC, 1)
    wk_c = wk[:, hs].astype(ml_dtypes.bfloat16)
    bk_c = bk[hs].astype(np.float32).reshape(DHC, 1)
    wv_aug = np.zeros((D, VW), np.float32)
    bvb = np.zeros((128, VW), np.float32)
    for h in range(H_CORE):
        c0 = 2 * HD * h
        wv_aug[:, c0:c0 + HD] = wv[:, DHC * g + HD * h:DHC * g + HD * (h + 1)]
        bvb[:, c0:c0 + HD] = bv[DHC * g + HD * h:DHC * g + HD * (h + 1)][None, :]
        bvb[:, c0 + HD:c0 + 2 * HD] = 1.0
    wo_c = wo[hs, :].astype(ml_dtypes.bfloat16)
    return {
        "xT": xT,
        "wq": wq_c, "bq": bq_c,
        "wk": wk_c, "bk": bk_c,
        "wv": wv_aug.astype(ml_dtypes.bfloat16), "bvb": bvb,
        "wo": wo_c,
    }


def kernel(x, wq, bq, wk, bk, wv, bv, wo, bo):
    x = np.asarray(x, np.float32)
    wq, bq = np.asarray(wq, np.float32), np.asarray(bq, np.float32)
    wk, bk = np.asarray(wk, np.float32), np.asarray(bk, np.float32)
    wv, bv = np.asarray(wv, np.float32), np.asarray(bv, np.float32)
    wo, bo = np.asarray(wo, np.float32), np.asarray(bo, np.float32)

    if "nc" not in _CACHE:
        _CACHE["nc"] = build_nc()
    nc = _CACHE["nc"]

    in_maps = []
    for c in range(N_CORES):
        b, g = divmod(c, 2)
        in_maps.append(_prep_core(x, wq, bq, wk, bk, wv, bv, wo, bo, b, g))

    res = run_bass_kernel_spmd(nc, in_maps, list(range(N_CORES)))

    out = np.empty((B, S, D), np.float32)
    for b in range(B):
        out[b] = (res.results[2 * b]["out"] + res.results[2 * b + 1]["out"]
                  + bo[None, :])
    return out


# revision 15
# speedup vs baseline: 1.0010x; 1.0010x over previous
"""Multi-head attention (B=4, S=2048, D=512, H=8) on 8 trn2 NeuronCores.

Sharding: core c handles batch b = c//2 and head-group g = c%2 (4 heads,
256 of the 512 model dims). Each core computes its 4 heads' attention and
a partial out-projection [2048, 512]; the host sums the two partials per
batch and adds the output bias.

Device kernel per core (all matmuls bf16 -> f32 PSUM):
  1. QKV projections from pre-transposed xT [512, 2048]:
       Q^T/K^T stored per head, zero-padded from 64 to 128 rows so the
       attention matmuls contract over K=128 (a K=64 matmul leaves half
       the PE array inactive and the HAM clock-gate then never grants
       2.4 GHz; the zero rows are numerically inert).
       V [128, 260] per seq-tile, with a per-head all-ones column
       (injected via the bias) so the P@V matmul also produces softmax
       row-sums. wq/bq are pre-scaled by 1/8 on the host.
  2. Per (q-block, head), flash-style: S^T tile [128, 1024] = K_h^T.Q_h,
     exp on ScalarE (PSUM -> SBUF bf16, double-buffered, software
     pipelined), P^T accumulated into O^T [65, 512] over 16 k-tiles.
     ScalarE (exp) is the saturated engine; the Q/K projection of heads
     2-3 and the out-projection of finished q-blocks are interleaved
     into the PE's slack via an aux work queue.
  3. Normalize: reciprocal of the row-sum row + gpsimd partition
     broadcast + multiply, all off the PE/ACT critical path.
  4. Out-projection per seq-tile: sum_p OT[p](chunk) @ wo[p].
No max-subtraction in softmax: scores are O(1) by construction, exp is
safe, and the reference softmax is shift-invariant.
"""

from collections import deque

import numpy as np
import ml_dtypes

import concourse.bacc as bacc
import concourse.mybir as mybir
from concourse.tile import TileContext
from concourse.bass_utils import run_bass_kernel_spmd

BF16 = mybir.dt.bfloat16
F32 = mybir.dt.float32
AF = mybir.ActivationFunctionType
ALU = mybir.AluOpType

B, S, D = 4, 2048, 512
H_CORE, HD = 4, 64          # heads per core, head dim
DHC = H_CORE * HD           # 256 dims per core
VW = H_CORE * 2 * HD        # 512: V augmented with 64 ones-columns per head
                            # (PV matmul then emits the softmax row-sum
                            # pre-replicated on partitions 64-127, and runs
                            # the full M=128 array)
N_CORES = 8

_CACHE = {}


def build_nc():
    nc = bacc.Bacc("TRN2", target_bir_lowering=False, debug=False,
                   num_devices=N_CORES)

    xT_d = nc.declare_dram_parameter("xT", [D, S], BF16, isOutput=False)
    wq_d = nc.declare_dram_parameter("wq", [D, DHC], BF16, isOutput=False)
    wk_d = nc.declare_dram_parameter("wk", [D, DHC], BF16, isOutput=False)
    wv_d = nc.declare_dram_parameter("wv", [D, VW], BF16, isOutput=False)
    wo_d = nc.declare_dram_parameter("wo", [DHC, D], BF16, isOutput=False)
    bq_d = nc.declare_dram_parameter("bq", [DHC, 1], F32, isOutput=False)
    bk_d = nc.declare_dram_parameter("bk", [DHC, 1], F32, isOutput=False)
    bvb_d = nc.declare_dram_parameter("bvb", [128, VW], F32, isOutput=False)
    out0_d = nc.declare_dram_parameter("out0", [S, D], F32, isOutput=True)
    out1_d = nc.declare_dram_parameter("out1", [S, D], F32, isOutput=True)
    out_ds = [out0_d, out1_d]

    NQB = 2          # q blocks of 1024
    QB = 1024
    NKT = S // 128   # 16 k tiles

    with TileContext(nc, num_cores=N_CORES) as tc:
        with (
            tc.tile_pool(name="persist", bufs=1) as pp,
            tc.tile_pool(name="pt_pool", bufs=3) as ptp,
            tc.tile_pool(name="rs_pool", bufs=2) as rsp,
            tc.tile_pool(name="ob_pool", bufs=3) as obp,
        ):
            # preload the exp ACT table before anything else: the first
            # real exp otherwise pays a ~2.7us table load that stalls the
            # whole pipeline
            scr = pp.tile([1, 8], F32, tag="scr", name="scr")
            nc.vector.memset(scr[:], 0.0)
            nc.scalar.activation(scr[:], scr[:], AF.Exp)

            # ---- load inputs (Q/K-proj operands first) ----
            xT = [pp.tile([128, S], BF16, tag=f"xT{i}", name=f"xT{i}") for i in range(4)]
            wq = [pp.tile([128, DHC], BF16, tag=f"wq{i}", name=f"wq{i}") for i in range(4)]
            wk = [pp.tile([128, DHC], BF16, tag=f"wk{i}", name=f"wk{i}") for i in range(4)]
            wv = [pp.tile([128, VW], BF16, tag=f"wv{i}", name=f"wv{i}") for i in range(4)]
            bq = [pp.tile([128, 1], F32, tag=f"bq{p}", name=f"bq{p}") for p in range(2)]
            bk = [pp.tile([128, 1], F32, tag=f"bk{p}", name=f"bk{p}") for p in range(2)]
            for i in range(4):
                nc.sync.dma_start(out=xT[i][:], in_=xT_d[128 * i:128 * (i + 1), :])
                nc.sync.dma_start(out=wq[i][:], in_=wq_d[128 * i:128 * (i + 1), :])
                nc.sync.dma_start(out=wk[i][:], in_=wk_d[128 * i:128 * (i + 1), :])
            for p in range(2):
                nc.sync.dma_start(out=bq[p][:], in_=bq_d[128 * p:128 * (p + 1), :])
                nc.sync.dma_start(out=bk[p][:], in_=bk_d[128 * p:128 * (p + 1), :])
            for i in range(4):
                nc.sync.dma_start(out=wv[i][:], in_=wv_d[128 * i:128 * (i + 1), :])
            bvb = pp.tile([128, VW], F32, tag="bvb")
            nc.sync.dma_start(out=bvb[:], in_=bvb_d[:])
            wo = [pp.tile([128, D], BF16, tag=f"wo{p}", name=f"wo{p}") for p in range(2)]
            for p in range(2):
                nc.sync.dma_start(out=wo[p][:], in_=wo_d[128 * p:128 * (p + 1), :])

            QTh = [pp.tile([128, S], BF16, tag=f"QTh{h}", name=f"QTh{h}")
                   for h in range(H_CORE)]
            KTh = [pp.tile([128, S], BF16, tag=f"KTh{h}", name=f"KTh{h}")
                   for h in range(H_CORE)]
            OT = [pp.tile([128, S], BF16, tag=f"OT{p}", name=f"OT{p}") for p in range(2)]
            V = [pp.tile([128, VW], BF16, tag=f"V{st}", name=f"V{st}") for st in range(NKT)]
            for h in range(H_CORE):
                nc.vector.memset(QTh[h][64:128, :], 0.0)
                nc.vector.memset(KTh[h][64:128, :], 0.0)

            # ---- Q / K projections for heads 0/1 (dout tile 0) ----
            # heads 2/3 (dout tile 1) are deferred into the attention
            # phase as aux work
            with tc.tile_pool(name="qk_ps", bufs=2, space="PSUM") as qkps:
                for (w_sb, b_sb, dst) in ((wq, bq, QTh), (wk, bk, KTh)):
                    ps = qkps.tile([128, S], F32, tag="qk", name="qk")
                    for din in range(4):
                        for st in range(4):
                            nc.tensor.matmul(
                                ps[:, 512 * st:512 * (st + 1)],
                                w_sb[din][:, 0:128],
                                xT[din][:, 512 * st:512 * (st + 1)],
                                start=(din == 0), stop=(din == 3),
                            )
                    for m in range(2):
                        nc.vector.tensor_scalar(
                            out=dst[m][0:64, :],
                            in0=ps[64 * m:64 * (m + 1), :],
                            scalar1=b_sb[0][64 * m:64 * (m + 1), :],
                            scalar2=None, op0=ALU.add,
                        )

            # ---- attention + interleaved aux work ----
            # aux queue: (min_kt, closure) emitted into the PE slack of the
            # ACT-bound attention loop. Carries the V projection (block 0),
            # the Q/K projection of heads 2-3 (block 1), and the finished
            # q-block's out-projection.
            with (
                tc.tile_pool(name="s_ps", bufs=2, space="PSUM") as sps,
                tc.tile_pool(name="o_ps", bufs=3, space="PSUM") as ops,
                tc.tile_pool(name="aux_ps", bufs=1, space="PSUM") as axp,
            ):
                aux = deque()

                def v_proj(st):
                    def run():
                        ps = axp.tile([128, 512], F32, tag="aux", name="aux")
                        for din in range(4):
                            nc.tensor.matmul(
                                ps[:, 0:VW],
                                xT[din][:, 128 * st:128 * (st + 1)],
                                wv[din][:],
                                start=(din == 0), stop=(din == 3),
                            )
                        nc.vector.tensor_tensor(
                            out=V[st][:], in0=ps[:, 0:VW], in1=bvb[:],
                            op=ALU.add)
                    return run

                def qk_dt1_aux(w_sb, b_sb, dst, st):
                    def run():
                        ps = axp.tile([128, 512], F32, tag="aux", name="aux")
                        for din in range(4):
                            nc.tensor.matmul(
                                ps[:],
                                w_sb[din][:, 128:256],
                                xT[din][:, 512 * st:512 * (st + 1)],
                                start=(din == 0), stop=(din == 3),
                            )
                        for m in range(2):
                            nc.vector.tensor_scalar(
                                out=dst[2 + m][0:64, 512 * st:512 * (st + 1)],
                                in0=ps[64 * m:64 * (m + 1), :],
                                scalar1=b_sb[1][64 * m:64 * (m + 1), :],
                                scalar2=None, op0=ALU.add,
                            )
                    return run

                def outproj_aux(p, st, pool):
                    def run():
                        ps = pool.tile([128, 512], F32, tag="aux",
                                       name="aux")
                        nc.tensor.matmul(
                            ps[:],
                            OT[p][:, 128 * st:128 * (st + 1)],
                            wo[p][:],
                            start=True, stop=True,
                        )
                        ob = obp.tile([128, D], F32, tag="ob", name="ob")
                        nc.vector.tensor_copy(ob[:], ps[:])
                        nc.sync.dma_start(
                            out=out_ds[p][128 * st:128 * (st + 1), :],
                            in_=ob[:])
                    run.light = True
                    run.p, run.st = p, st
                    return run

                # V[0..1] inline (needed by the first PV steps of block 0);
                # the rest rides the aux queue one step ahead of use
                v_proj(0)()
                v_proj(1)()
                for st in range(2, NKT):
                    aux.append((st - 1, v_proj(st)))
                for st in range(4):
                    aux.append((1, qk_dt1_aux(wq, bq, QTh, st)))
                    aux.append((1, qk_dt1_aux(wk, bk, KTh, st)))

                def block(h, qb):
                    p, m = divmod(h, 2)
                    r0, r1 = 64 * m, 64 * (m + 1)
                    q0 = QB * qb
                    o_acc = []

                    def s_mms(kt):
                        stile = sps.tile([128, QB], F32, tag="s", name="s")
                        for qt in range(2):
                            nc.tensor.matmul(
                                stile[:, 512 * qt:512 * (qt + 1)],
                                KTh[h][:, 128 * kt:128 * (kt + 1)],
                                QTh[h][:, q0 + 512 * qt:q0 + 512 * (qt + 1)],
                                start=True, stop=True,
                            )
                        return stile

                    def exp_pv(kt, stile):
                        pt = ptp.tile([128, QB], BF16, tag="pt", name="pt")
                        nc.scalar.activation(pt[:], stile[:], AF.Exp)
                        for qt in range(2):
                            nc.tensor.matmul(
                                o_acc[qt][:],
                                V[kt][:, 2 * HD * h:2 * HD * (h + 1)],
                                pt[:, 512 * qt:512 * (qt + 1)],
                                start=(kt == 0), stop=(kt == NKT - 1),
                            )

                    prev = s_mms(0)
                    for kt in range(1, NKT):
                        cur = s_mms(kt)
                        if not o_acc:
                            o_acc.extend(
                                ops.tile([128, 512], F32, tag="o",
                                         name="o_acc") for _ in range(2))
                        popped = 0
                        while (aux and kt >= aux[0][0] and popped <
                               (2 if getattr(aux[0][1], "light", False)
                                else 1)):
                            aux.popleft()[1]()
                            popped += 1
                        exp_pv(kt - 1, prev)
                        prev = cur
                    exp_pv(NKT - 1, prev)

                    # free the o_acc PSUM slots with one fast copy each;
                    # rows 64-127 already hold the row-sum replicated, so
                    # normalize = reciprocal + same-base multiply, all DVE
                    osb = []
                    for qt in range(2):
                        t = rsp.tile([128, 512], F32, tag="osb", name="osb")
                        nc.vector.tensor_copy(t[:], o_acc[qt][:])
                        osb.append(t)
                    for qt in range(2):
                        recB = rsp.tile([HD, 512], F32, tag="recB",
                                        name="recB")
                        nc.vector.reciprocal(recB[:], osb[qt][HD:2 * HD, :])
                        nc.vector.tensor_tensor(
                            out=OT[p][r0:r1,
                                      q0 + 512 * qt:q0 + 512 * (qt + 1)],
                            in0=osb[qt][0:HD, :], in1=recB[:],
                            op=ALU.mult,
                        )

                MKT = [10, 10, 11, 11, 12, 12, 13, 13]
                for qb in range(NQB):
                    for h in range(H_CORE):
                        block(h, qb)
                        # pair p's OT columns for this q-block are final
                        # one block after its second head's epilogue
                        if h % 2 == 1:
                            p = h // 2
                            for j, st in enumerate(
                                    range(8 * qb, 8 * (qb + 1))):
                                aux.append((MKT[j], outproj_aux(p, st, axp)))

            # drain the remaining out-projections (last pair of the last
            # q-block) with a deeper pool now that the attention PSUM is free
            with tc.tile_pool(name="tail_ps", bufs=4, space="PSUM") as tlp:
                while aux:
                    _, it = aux.popleft()
                    outproj_aux(it.p, it.st, tlp)()

    nc.compile()
    return nc


def _prep_core(x, wq, bq, wk, bk, wv, bv, wo, bo, b, g):
    hs = slice(DHC * g, DHC * (g + 1))
    xT = np.ascontiguousarray(x[b].T).astype(ml_dtypes.bfloat16)
    wq_c = (wq[:, hs] / 8.0).astype(ml_dtypes.bfloat16)
    bq_c = (bq[hs] / 8.0).astype(np.float32).reshape(DHC, 1)
    wk_c = wk[:, hs].astype(ml_dtypes.bfloat16)
    bk_c = bk[hs].astype(np.float32).reshape(DHC, 1)
    wv_aug = np.zeros((D, VW), np.float32)
    bvb = np.zeros((128, VW), np.float32)
    for h in range(H_CORE):
        c0 = 2 * HD * h
        wv_aug[:, c0:c0 + HD] = wv[:, DHC * g + HD * h:DHC * g + HD * (h + 1)]
        bvb[:, c0:c0 + HD] = bv[DHC * g + HD * h:DHC * g + HD * (h + 1)][None, :]
        bvb[:, c0 + HD:c0 + 2 * HD] = 1.0
    wo_c = wo[hs, :].astype(ml_dtypes.bfloat16)
    return {
        "xT": xT,
        "wq": wq_c, "bq": bq_c,
        "wk": wk_c, "bk": bk_c,
        "wv": wv_aug.astype(ml_dtypes.bfloat16), "bvb": bvb,
        "wo": wo_c,
    }


def kernel(x, wq, bq, wk, bk, wv, bv, wo, bo):
    x = np.asarray(x, np.float32)
    wq, bq = np.asarray(wq, np.float32), np.asarray(bq, np.float32)
    wk, bk = np.asarray(wk, np.float32), np.asarray(bk, np.float32)
    wv, bv = np.asarray(wv, np.float32), np.asarray(bv, np.float32)
    wo, bo = np.asarray(wo, np.float32), np.asarray(bo, np.float32)

    if "nc" not in _CACHE:
        _CACHE["nc"] = build_nc()
    nc = _CACHE["nc"]

    in_maps = []
    for c in range(N_CORES):
        b, g = divmod(c, 2)
        in_maps.append(_prep_core(x, wq, bq, wk, bk, wv, bv, wo, bo, b, g))

    res = run_bass_kernel_spmd(nc, in_maps, list(range(N_CORES)))

    out = np.empty((B, S, D), np.float32)
    for b in range(B):
        r0, r1 = res.results[2 * b], res.results[2 * b + 1]
        out[b] = (r0["out0"] + r0["out1"] + r1["out0"] + r1["out1"]
                  + bo[None, :])
    return out


# revision 18
# speedup vs baseline: 1.0365x; 1.0354x over previous
"""Multi-head attention (B=4, S=2048, D=512, H=8) on 8 trn2 NeuronCores.

Sharding: core c handles batch b = c//2 and head-group g = c%2 (4 heads,
256 of the 512 model dims). Each core computes its 4 heads' attention and
a partial out-projection [2048, 512]; the host sums the two partials per
batch and adds the output bias.

Device kernel per core (all matmuls bf16 -> f32 PSUM):
  1. QKV projections from pre-transposed xT [512, 2048]:
       Q^T/K^T stored per head, zero-padded from 64 to 128 rows so the
       attention matmuls contract over K=128 (a K=64 matmul leaves half
       the PE array inactive and the HAM clock-gate then never grants
       2.4 GHz; the zero rows are numerically inert).
       V [128, 512] per seq-tile with 64 all-ones columns per head
       (injected via the bias) so the P@V matmul emits the softmax
       row-sum pre-replicated on its partitions 64-127 and runs the
       full M=128 array. wq/bq are pre-scaled by 1/8 on the host.
  2. Per (q-block, head), flash-style: S^T tile [128, 1024] = K_h^T.Q_h,
     exp on ScalarE (PSUM -> SBUF bf16, double-buffered, software
     pipelined), P^T accumulated into O^T [128, 512] over 16 k-tiles.
     ScalarE (exp) is the saturated engine; everything else (V
     projection, heads 2-3's Q/K projection, normalize, out-projection)
     rides a priority work queue drained into the loop's slack.
  3. Normalize per q-tile: DVE reciprocal of the replicated row-sum
     block + same-base multiply, deferred one block so its DVE burst
     never lands on a block boundary.
No max-subtraction in softmax: scores are O(1) by construction, exp is
safe, and the reference softmax is shift-invariant.
"""

import numpy as np
import ml_dtypes

import concourse.bacc as bacc
import concourse.mybir as mybir
from concourse.tile import TileContext
from concourse.bass_utils import run_bass_kernel_spmd

BF16 = mybir.dt.bfloat16
F32 = mybir.dt.float32
AF = mybir.ActivationFunctionType
ALU = mybir.AluOpType

B, S, D = 4, 2048, 512
H_CORE, HD = 4, 64          # heads per core, head dim
DHC = H_CORE * HD           # 256 dims per core
VW = H_CORE * 2 * HD        # 512: V augmented with 64 ones-columns per head
N_CORES = 8

_CACHE = {}


def build_nc():
    nc = bacc.Bacc("TRN2", target_bir_lowering=False, debug=False,
                   num_devices=N_CORES)

    xT_d = nc.declare_dram_parameter("xT", [D, S], BF16, isOutput=False)
    wq_d = nc.declare_dram_parameter("wq", [D, DHC], BF16, isOutput=False)
    wk_d = nc.declare_dram_parameter("wk", [D, DHC], BF16, isOutput=False)
    wv_d = nc.declare_dram_parameter("wv", [D, VW], BF16, isOutput=False)
    wo_d = nc.declare_dram_parameter("wo", [DHC, D], BF16, isOutput=False)
    bq_d = nc.declare_dram_parameter("bq", [DHC, 1], F32, isOutput=False)
    bk_d = nc.declare_dram_parameter("bk", [DHC, 1], F32, isOutput=False)
    bvb_d = nc.declare_dram_parameter("bvb", [128, VW], F32, isOutput=False)
    out_d = nc.declare_dram_parameter("out", [S, D], F32, isOutput=True)

    NQB = 2          # q blocks of 1024
    QB = 1024
    NKT = S // 128   # 16 k tiles

    with TileContext(nc, num_cores=N_CORES) as tc:
        with (
            tc.tile_pool(name="persist", bufs=1) as pp,
            tc.tile_pool(name="pt_pool", bufs=3) as ptp,
            tc.tile_pool(name="rs_pool", bufs=2) as rsp,
            tc.tile_pool(name="ob_pool", bufs=3) as obp,
        ):
            # preload the exp ACT table before anything else: the first
            # real exp otherwise pays a ~2.7us table load that stalls the
            # whole pipeline
            scr = pp.tile([1, 8], F32, tag="scr", name="scr")
            nc.vector.memset(scr[:], 0.0)
            nc.scalar.activation(scr[:], scr[:], AF.Exp)

            # ---- load inputs (Q/K-proj operands first) ----
            xT = [pp.tile([128, S], BF16, tag=f"xT{i}", name=f"xT{i}")
                  for i in range(4)]
            wq = [pp.tile([128, DHC], BF16, tag=f"wq{i}", name=f"wq{i}")
                  for i in range(4)]
            wk = [pp.tile([128, DHC], BF16, tag=f"wk{i}", name=f"wk{i}")
                  for i in range(4)]
            wv = [pp.tile([128, VW], BF16, tag=f"wv{i}", name=f"wv{i}")
                  for i in range(4)]
            bq = [pp.tile([128, 1], F32, tag=f"bq{p}", name=f"bq{p}")
                  for p in range(2)]
            bk = [pp.tile([128, 1], F32, tag=f"bk{p}", name=f"bk{p}")
                  for p in range(2)]
            for i in range(4):
                nc.sync.dma_start(out=xT[i][:], in_=xT_d[128 * i:128 * (i + 1), :])
                nc.sync.dma_start(out=wq[i][:], in_=wq_d[128 * i:128 * (i + 1), :])
                nc.sync.dma_start(out=wk[i][:], in_=wk_d[128 * i:128 * (i + 1), :])
            for p in range(2):
                nc.sync.dma_start(out=bq[p][:], in_=bq_d[128 * p:128 * (p + 1), :])
                nc.sync.dma_start(out=bk[p][:], in_=bk_d[128 * p:128 * (p + 1), :])
            for i in range(4):
                nc.sync.dma_start(out=wv[i][:], in_=wv_d[128 * i:128 * (i + 1), :])
            bvb = pp.tile([128, VW], F32, tag="bvb")
            nc.sync.dma_start(out=bvb[:], in_=bvb_d[:])
            wo = [pp.tile([128, D], BF16, tag=f"wo{p}", name=f"wo{p}")
                  for p in range(2)]
            for p in range(2):
                nc.sync.dma_start(out=wo[p][:], in_=wo_d[128 * p:128 * (p + 1), :])

            QTh = [pp.tile([128, S], BF16, tag=f"QTh{h}", name=f"QTh{h}")
                   for h in range(H_CORE)]
            KTh = [pp.tile([128, S], BF16, tag=f"KTh{h}", name=f"KTh{h}")
                   for h in range(H_CORE)]
            OT = [pp.tile([128, S], BF16, tag=f"OT{p}", name=f"OT{p}")
                  for p in range(2)]
            V = [pp.tile([128, VW], BF16, tag=f"V{st}", name=f"V{st}")
                 for st in range(NKT)]
            for h in range(H_CORE):
                nc.vector.memset(QTh[h][64:128, :], 0.0)
                nc.vector.memset(KTh[h][64:128, :], 0.0)

            # ---- Q / K projections for heads 0/1 (dout tile 0) ----
            with tc.tile_pool(name="qk_ps", bufs=2, space="PSUM") as qkps:
                for (w_sb, b_sb, dst) in ((wq, bq, QTh), (wk, bk, KTh)):
                    ps = qkps.tile([128, S], F32, tag="qk", name="qk")
                    for din in range(4):
                        for st in range(4):
                            nc.tensor.matmul(
                                ps[:, 512 * st:512 * (st + 1)],
                                w_sb[din][:, 0:128],
                                xT[din][:, 512 * st:512 * (st + 1)],
                                start=(din == 0), stop=(din == 3),
                            )
                    for m in range(2):
                        nc.vector.tensor_scalar(
                            out=dst[m][0:64, :],
                            in0=ps[64 * m:64 * (m + 1), :],
                            scalar1=b_sb[0][64 * m:64 * (m + 1), :],
                            scalar2=None, op0=ALU.add,
                        )

            # ---- attention + priority work queue ----
            # queue items: (min_kt, cost, fn); a per-step budget of 2 is
            # drained smallest-min_kt-first into the PE slack
            with (
                tc.tile_pool(name="s_ps", bufs=2, space="PSUM") as sps,
                tc.tile_pool(name="o_ps", bufs=3, space="PSUM") as ops,
                tc.tile_pool(name="aux_ps", bufs=1, space="PSUM") as axp,
            ):
                aux = []

                def v_proj(st):
                    def run():
                        ps = axp.tile([128, 512], F32, tag="aux", name="aux")
                        for din in range(4):
                            nc.tensor.matmul(
                                ps[:, 0:VW],
                                xT[din][:, 128 * st:128 * (st + 1)],
                                wv[din][:],
                                start=(din == 0), stop=(din == 3),
                            )
                        nc.vector.tensor_tensor(
                            out=V[st][:], in0=ps[:, 0:VW], in1=bvb[:],
                            op=ALU.add)
                    return run

                def qk_dt1_aux(w_sb, b_sb, dst, st):
                    def run():
                        ps = axp.tile([128, 512], F32, tag="aux", name="aux")
                        for din in range(4):
                            nc.tensor.matmul(
                                ps[:],
                                w_sb[din][:, 128:256],
                                xT[din][:, 512 * st:512 * (st + 1)],
                                start=(din == 0), stop=(din == 3),
                            )
                        for m in range(2):
                            nc.vector.tensor_scalar(
                                out=dst[2 + m][0:64, 512 * st:512 * (st + 1)],
                                in0=ps[64 * m:64 * (m + 1), :],
                                scalar1=b_sb[1][64 * m:64 * (m + 1), :],
                                scalar2=None, op0=ALU.add,
                            )
                    return run

                def outproj(st, pool):
                    def run():
                        ps = pool.tile([128, 512], F32, tag="aux",
                                       name="aux")
                        for p in range(2):
                            nc.tensor.matmul(
                                ps[:],
                                OT[p][:, 128 * st:128 * (st + 1)],
                                wo[p][:],
                                start=(p == 0), stop=(p == 1),
                            )
                        ob = obp.tile([128, D], F32, tag="ob", name="ob")
                        nc.vector.tensor_copy(ob[:], ps[:])
                        nc.sync.dma_start(
                            out=out_d[128 * st:128 * (st + 1), :], in_=ob[:])
                    return run

                # V[0..1] inline (needed by the first PV steps of block 0);
                # the rest rides the queue one step ahead of use
                v_proj(0)()
                v_proj(1)()
                for st in range(2, NKT):
                    aux.append((st - 1, 2, v_proj(st)))
                for st in range(4):
                    aux.append((1, 2, qk_dt1_aux(wq, bq, QTh, st)))
                    aux.append((1, 2, qk_dt1_aux(wk, bk, KTh, st)))

                def block(h, qb):
                    p, m = divmod(h, 2)
                    r0, r1 = 64 * m, 64 * (m + 1)
                    q0 = QB * qb
                    o_acc = []

                    def s_mms(kt):
                        stile = sps.tile([128, QB], F32, tag="s", name="s")
                        for qt in range(2):
                            nc.tensor.matmul(
                                stile[:, 512 * qt:512 * (qt + 1)],
                                KTh[h][:, 128 * kt:128 * (kt + 1)],
                                QTh[h][:, q0 + 512 * qt:q0 + 512 * (qt + 1)],
                                start=True, stop=True,
                            )
                        return stile

                    def exp_pv(kt, stile):
                        pt = ptp.tile([128, QB], BF16, tag="pt", name="pt")
                        nc.scalar.activation(pt[:], stile[:], AF.Exp)
                        for qt in range(2):
                            nc.tensor.matmul(
                                o_acc[qt][:],
                                V[kt][:, 2 * HD * h:2 * HD * (h + 1)],
                                pt[:, 512 * qt:512 * (qt + 1)],
                                start=(kt == 0), stop=(kt == NKT - 1),
                            )

                    prev = s_mms(0)
                    for kt in range(1, NKT):
                        cur = s_mms(kt)
                        if not o_acc:
                            o_acc.extend(
                                ops.tile([128, 512], F32, tag="o",
                                         name="o_acc") for _ in range(2))
                        # strict FIFO: enqueue order encodes producer ->
                        # consumer program order; min_kt only gates timing
                        budget = 2
                        while (aux and aux[0][0] <= kt
                               and aux[0][1] <= budget):
                            _, c, fn = aux.pop(0)
                            fn()
                            budget -= c
                        exp_pv(kt - 1, prev)
                        prev = cur
                    exp_pv(NKT - 1, prev)

                    # free the o_acc PSUM slots with one fast DVE copy
                    # each; the slow reciprocal+multiply is deferred into
                    # the next block via the queue
                    osb = []
                    for qt in range(2):
                        t = rsp.tile([128, 512], F32, tag="osb", name="osb")
                        nc.vector.tensor_copy(t[:], o_acc[qt][:])
                        osb.append(t)

                    def normalize(qt):
                        def run():
                            recB = rsp.tile([HD, 512], F32, tag="recB",
                                            name="recB")
                            nc.vector.reciprocal(
                                recB[:], osb[qt][HD:2 * HD, :])
                            nc.vector.tensor_tensor(
                                out=OT[p][r0:r1,
                                          q0 + 512 * qt:q0 + 512 * (qt + 1)],
                                in0=osb[qt][0:HD, :], in1=recB[:],
                                op=ALU.mult,
                            )
                        return run
                    return [normalize(0), normalize(1)]

                MKT = [10, 10, 11, 11, 12, 12, 13, 13]
                last_norm = None
                for qb in range(NQB):
                    for h in range(H_CORE):
                        norms = block(h, qb)
                        if (qb, h) == (NQB - 1, H_CORE - 1):
                            last_norm = norms
                        else:
                            for i, nrm in enumerate(norms):
                                aux.append((1 + i, 1, nrm))
                    # out-projection of this q-block: needs every head's
                    # normalize, which lands early in the following block
                    if qb == 0:
                        for j, st in enumerate(range(8)):
                            aux.append((MKT[j], 2, outproj(st, axp)))
                tail_sts = list(range(8, 16))
                leftovers = sorted(aux, key=lambda x: x[0])

            # tail: drain with a deeper pool; interleave the last block's
            # normalize with the out-projections that don't depend on it
            with tc.tile_pool(name="tail_ps", bufs=4, space="PSUM") as tlp:
                for _, _, fn in leftovers:
                    fn()
                last_norm[0]()
                for st in tail_sts[:4]:
                    outproj(st, tlp)()
                last_norm[1]()
                for st in tail_sts[4:]:
                    outproj(st, tlp)()

    nc.compile()
    return nc


def _prep_core(x, wq, bq, wk, bk, wv, bv, wo, bo, b, g):
    hs = slice(DHC * g, DHC * (g + 1))
    xT = np.ascontiguousarray(x[b].T).astype(ml_dtypes.bfloat16)
    wq_c = (wq[:, hs] / 8.0).astype(ml_dtypes.bfloat16)
    bq_c = (bq[hs] / 8.0).astype(np.float32).reshape(DHC, 1)
    wk_c = wk[:, hs].astype(ml_dtypes.bfloat16)
    bk_c = bk[hs].astype(np.float32).reshape(DHC, 1)
    wv_aug = np.zeros((D, VW), np.float32)
    bvb = np.zeros((128, VW), np.float32)
    for h in range(H_CORE):
        c0 = 2 * HD * h
        wv_aug[:, c0:c0 + HD] = wv[:, DHC * g + HD * h:DHC * g + HD * (h + 1)]
        bvb[:, c0:c0 + HD] = bv[DHC * g + HD * h:DHC * g + HD * (h + 1)][None, :]
        bvb[:, c0 + HD:c0 + 2 * HD] = 1.0
    wo_c = wo[hs, :].astype(ml_dtypes.bfloat16)
    return {
        "xT": xT,
        "wq": wq_c, "bq": bq_c,
        "wk": wk_c, "bk": bk_c,
        "wv": wv_aug.astype(ml_dtypes.bfloat16), "bvb": bvb,
        "wo": wo_c,
    }


def kernel(x, wq, bq, wk, bk, wv, bv, wo, bo):
    x = np.asarray(x, np.float32)
    wq, bq = np.asarray(wq, np.float32), np.asarray(bq, np.float32)
    wk, bk = np.asarray(wk, np.float32), np.asarray(bk, np.float32)
    wv, bv = np.asarray(wv, np.float32), np.asarray(bv, np.float32)
    wo, bo = np.asarray(wo, np.float32), np.asarray(bo, np.float32)

    if "nc" not in _CACHE:
        _CACHE["nc"] = build_nc()
    nc = _CACHE["nc"]

    in_maps = []
    for c in range(N_CORES):
        b, g = divmod(c, 2)
        in_maps.append(_prep_core(x, wq, bq, wk, bk, wv, bv, wo, bo, b, g))

    res = run_bass_kernel_spmd(nc, in_maps, list(range(N_CORES)))

    out = np.empty((B, S, D), np.float32)
    for b in range(B):
        out[b] = (res.results[2 * b]["out"] + res.results[2 * b + 1]["out"]
                  + bo[None, :])
    return out


# revision 19
# speedup vs baseline: 1.0379x; 1.0014x over previous
"""Multi-head attention (B=4, S=2048, D=512, H=8) on 8 trn2 NeuronCores.

Sharding: core c handles batch b = c//2 and head-group g = c%2 (4 heads,
256 of the 512 model dims). Each core computes its 4 heads' attention and
a partial out-projection [2048, 512]; the host sums the two partials per
batch and adds the output bias.

Device kernel per core (all matmuls bf16 -> f32 PSUM):
  1. QKV projections from pre-transposed xT [512, 2048]:
       Q^T/K^T stored per head, zero-padded from 64 to 128 rows so the
       attention matmuls contract over K=128 (a K=64 matmul leaves half
       the PE array inactive and the HAM clock-gate then never grants
       2.4 GHz; the zero rows are numerically inert).
       V [128, 512] per seq-tile with 64 all-ones columns per head
       (injected via the bias) so the P@V matmul emits the softmax
       row-sum pre-replicated on its partitions 64-127 and runs the
       full M=128 array. wq/bq are pre-scaled by 1/8 on the host.
  2. Per (q-block, head), flash-style: S^T tile [128, 1024] = K_h^T.Q_h,
     exp on ScalarE (PSUM -> SBUF bf16, double-buffered, software
     pipelined), P^T accumulated into O^T [128, 512] over 16 k-tiles.
     ScalarE (exp) is the saturated engine; everything else (V
     projection, heads 2-3's Q/K projection, normalize, out-projection)
     rides a priority work queue drained into the loop's slack.
  3. Normalize per q-tile: DVE reciprocal of the replicated row-sum
     block + same-base multiply, deferred one block so its DVE burst
     never lands on a block boundary.
No max-subtraction in softmax: scores are O(1) by construction, exp is
safe, and the reference softmax is shift-invariant.
"""

import numpy as np
import ml_dtypes

import concourse.bacc as bacc
import concourse.mybir as mybir
from concourse.tile import TileContext
from concourse.bass_utils import run_bass_kernel_spmd

BF16 = mybir.dt.bfloat16
F32 = mybir.dt.float32
AF = mybir.ActivationFunctionType
ALU = mybir.AluOpType

B, S, D = 4, 2048, 512
H_CORE, HD = 4, 64          # heads per core, head dim
DHC = H_CORE * HD           # 256 dims per core
VW = H_CORE * 2 * HD        # 512: V augmented with 64 ones-columns per head
N_CORES = 8

_CACHE = {}


def build_nc():
    nc = bacc.Bacc("TRN2", target_bir_lowering=False, debug=False,
                   num_devices=N_CORES)

    xT_d = nc.declare_dram_parameter("xT", [D, S], BF16, isOutput=False)
    wq_d = nc.declare_dram_parameter("wq", [D, DHC], BF16, isOutput=False)
    wk_d = nc.declare_dram_parameter("wk", [D, DHC], BF16, isOutput=False)
    wv_d = nc.declare_dram_parameter("wv", [D, VW], BF16, isOutput=False)
    wo_d = nc.declare_dram_parameter("wo", [DHC, D], BF16, isOutput=False)
    bq_d = nc.declare_dram_parameter("bq", [DHC, 1], F32, isOutput=False)
    bk_d = nc.declare_dram_parameter("bk", [DHC, 1], F32, isOutput=False)
    bvb_d = nc.declare_dram_parameter("bvb", [128, VW], F32, isOutput=False)
    out_d = nc.declare_dram_parameter("out", [S, D], F32, isOutput=True)

    NQB = 2          # q blocks of 1024
    QB = 1024
    NKT = S // 128   # 16 k tiles

    with TileContext(nc, num_cores=N_CORES) as tc:
        with (
            tc.tile_pool(name="persist", bufs=1) as pp,
            tc.tile_pool(name="pt_pool", bufs=3) as ptp,
            tc.tile_pool(name="rs_pool", bufs=2) as rsp,
            tc.tile_pool(name="ob_pool", bufs=3) as obp,
        ):
            # preload the exp ACT table before anything else: the first
            # real exp otherwise pays a ~2.7us table load that stalls the
            # whole pipeline
            scr = pp.tile([1, 8], F32, tag="scr", name="scr")
            nc.vector.memset(scr[:], 0.0)
            nc.scalar.activation(scr[:], scr[:], AF.Exp)

            # ---- load inputs (Q/K-proj operands first) ----
            xT = [pp.tile([128, S], BF16, tag=f"xT{i}", name=f"xT{i}")
                  for i in range(4)]
            wq = [pp.tile([128, DHC], BF16, tag=f"wq{i}", name=f"wq{i}")
                  for i in range(4)]
            wk = [pp.tile([128, DHC], BF16, tag=f"wk{i}", name=f"wk{i}")
                  for i in range(4)]
            wv = [pp.tile([128, VW], BF16, tag=f"wv{i}", name=f"wv{i}")
                  for i in range(4)]
            bq = [pp.tile([128, 1], F32, tag=f"bq{p}", name=f"bq{p}")
                  for p in range(2)]
            bk = [pp.tile([128, 1], F32, tag=f"bk{p}", name=f"bk{p}")
                  for p in range(2)]
            for i in range(4):
                nc.sync.dma_start(out=xT[i][:], in_=xT_d[128 * i:128 * (i + 1), :])
                nc.sync.dma_start(out=wq[i][:], in_=wq_d[128 * i:128 * (i + 1), :])
                nc.sync.dma_start(out=wk[i][:], in_=wk_d[128 * i:128 * (i + 1), :])
            for p in range(2):
                nc.sync.dma_start(out=bq[p][:], in_=bq_d[128 * p:128 * (p + 1), :])
                nc.sync.dma_start(out=bk[p][:], in_=bk_d[128 * p:128 * (p + 1), :])
            for i in range(4):
                nc.sync.dma_start(out=wv[i][:], in_=wv_d[128 * i:128 * (i + 1), :])
            bvb = pp.tile([128, VW], F32, tag="bvb")
            nc.sync.dma_start(out=bvb[:], in_=bvb_d[:])
            wo = [pp.tile([128, D], BF16, tag=f"wo{p}", name=f"wo{p}")
                  for p in range(2)]
            for p in range(2):
                nc.sync.dma_start(out=wo[p][:], in_=wo_d[128 * p:128 * (p + 1), :])

            QTh = [pp.tile([128, S], BF16, tag=f"QTh{h}", name=f"QTh{h}")
                   for h in range(H_CORE)]
            KTh = [pp.tile([128, S], BF16, tag=f"KTh{h}", name=f"KTh{h}")
                   for h in range(H_CORE)]
            OT = [pp.tile([128, S], BF16, tag=f"OT{p}", name=f"OT{p}")
                  for p in range(2)]
            V = [pp.tile([128, VW], BF16, tag=f"V{st}", name=f"V{st}")
                 for st in range(NKT)]
            for h in range(H_CORE):
                nc.vector.memset(QTh[h][64:128, :], 0.0)
                nc.vector.memset(KTh[h][64:128, :], 0.0)

            # ---- Q / K projections for heads 0/1 (dout tile 0) ----
            with tc.tile_pool(name="qk_ps", bufs=2, space="PSUM") as qkps:
                for (w_sb, b_sb, dst) in ((wq, bq, QTh), (wk, bk, KTh)):
                    ps = qkps.tile([128, S], F32, tag="qk", name="qk")
                    for din in range(4):
                        for st in range(4):
                            nc.tensor.matmul(
                                ps[:, 512 * st:512 * (st + 1)],
                                w_sb[din][:, 0:128],
                                xT[din][:, 512 * st:512 * (st + 1)],
                                start=(din == 0), stop=(din == 3),
                            )
                    for m in range(2):
                        nc.vector.tensor_scalar(
                            out=dst[m][0:64, :],
                            in0=ps[64 * m:64 * (m + 1), :],
                            scalar1=b_sb[0][64 * m:64 * (m + 1), :],
                            scalar2=None, op0=ALU.add,
                        )

            # ---- attention + priority work queue ----
            # queue items: (min_kt, cost, fn); a per-step budget of 2 is
            # drained smallest-min_kt-first into the PE slack
            with (
                tc.tile_pool(name="s_ps", bufs=2, space="PSUM") as sps,
                tc.tile_pool(name="o_ps", bufs=2, space="PSUM") as ops,
                tc.tile_pool(name="aux_ps", bufs=2, space="PSUM") as axp,
            ):
                aux = []

                def v_proj(st):
                    def run():
                        ps = axp.tile([128, 512], F32, tag="aux", name="aux")
                        for din in range(4):
                            nc.tensor.matmul(
                                ps[:, 0:VW],
                                xT[din][:, 128 * st:128 * (st + 1)],
                                wv[din][:],
                                start=(din == 0), stop=(din == 3),
                            )
                        nc.vector.tensor_tensor(
                            out=V[st][:], in0=ps[:, 0:VW], in1=bvb[:],
                            op=ALU.add)
                    return run

                def qk_dt1_aux(w_sb, b_sb, dst, st):
                    def run():
                        ps = axp.tile([128, 512], F32, tag="aux", name="aux")
                        for din in range(4):
                            nc.tensor.matmul(
                                ps[:],
                                w_sb[din][:, 128:256],
                                xT[din][:, 512 * st:512 * (st + 1)],
                                start=(din == 0), stop=(din == 3),
                            )
                        for m in range(2):
                            nc.vector.tensor_scalar(
                                out=dst[2 + m][0:64, 512 * st:512 * (st + 1)],
                                in0=ps[64 * m:64 * (m + 1), :],
                                scalar1=b_sb[1][64 * m:64 * (m + 1), :],
                                scalar2=None, op0=ALU.add,
                            )
                    return run

                def outproj(st, pool):
                    def run():
                        ps = pool.tile([128, 512], F32, tag="aux",
                                       name="aux")
                        for p in range(2):
                            nc.tensor.matmul(
                                ps[:],
                                OT[p][:, 128 * st:128 * (st + 1)],
                                wo[p][:],
                                start=(p == 0), stop=(p == 1),
                            )
                        ob = obp.tile([128, D], F32, tag="ob", name="ob")
                        nc.vector.tensor_copy(ob[:], ps[:])
                        nc.sync.dma_start(
                            out=out_d[128 * st:128 * (st + 1), :], in_=ob[:])
                    return run

                # V[0..1] inline (needed by the first PV steps of block 0);
                # the rest rides the queue one step ahead of use
                v_proj(0)()
                v_proj(1)()
                for st in range(2, NKT):
                    aux.append((st - 1, 2, v_proj(st)))
                for st in range(4):
                    aux.append((1, 2, qk_dt1_aux(wq, bq, QTh, st)))
                    aux.append((1, 2, qk_dt1_aux(wk, bk, KTh, st)))

                def block(h, qb):
                    p, m = divmod(h, 2)
                    r0, r1 = 64 * m, 64 * (m + 1)
                    q0 = QB * qb
                    o_acc = []

                    def s_mms(kt):
                        stile = sps.tile([128, QB], F32, tag="s", name="s")
                        for qt in range(2):
                            nc.tensor.matmul(
                                stile[:, 512 * qt:512 * (qt + 1)],
                                KTh[h][:, 128 * kt:128 * (kt + 1)],
                                QTh[h][:, q0 + 512 * qt:q0 + 512 * (qt + 1)],
                                start=True, stop=True,
                            )
                        return stile

                    def exp_pv(kt, stile):
                        pt = ptp.tile([128, QB], BF16, tag="pt", name="pt")
                        nc.scalar.activation(pt[:], stile[:], AF.Exp)
                        for qt in range(2):
                            nc.tensor.matmul(
                                o_acc[qt][:],
                                V[kt][:, 2 * HD * h:2 * HD * (h + 1)],
                                pt[:, 512 * qt:512 * (qt + 1)],
                                start=(kt == 0), stop=(kt == NKT - 1),
                            )

                    prev = s_mms(0)
                    for kt in range(1, NKT):
                        cur = s_mms(kt)
                        if not o_acc:
                            o_acc.extend(
                                ops.tile([128, 512], F32, tag="o",
                                         name="o_acc") for _ in range(2))
                        # strict FIFO: enqueue order encodes producer ->
                        # consumer program order; min_kt only gates timing
                        budget = 2
                        while (aux and aux[0][0] <= kt
                               and aux[0][1] <= budget):
                            _, c, fn = aux.pop(0)
                            fn()
                            budget -= c
                        exp_pv(kt - 1, prev)
                        prev = cur
                    exp_pv(NKT - 1, prev)

                    # free the o_acc PSUM slots with one fast DVE copy
                    # each; the slow reciprocal+multiply is deferred into
                    # the next block via the queue
                    osb = []
                    for qt in range(2):
                        t = rsp.tile([128, 512], F32, tag="osb", name="osb")
                        nc.vector.tensor_copy(t[:], o_acc[qt][:])
                        osb.append(t)

                    def normalize(qt):
                        def run():
                            recB = rsp.tile([HD, 512], F32, tag="recB",
                                            name="recB")
                            nc.vector.reciprocal(
                                recB[:], osb[qt][HD:2 * HD, :])
                            nc.vector.tensor_tensor(
                                out=OT[p][r0:r1,
                                          q0 + 512 * qt:q0 + 512 * (qt + 1)],
                                in0=osb[qt][0:HD, :], in1=recB[:],
                                op=ALU.mult,
                            )
                        return run
                    return [normalize(0), normalize(1)]

                MKT = [10, 10, 11, 11, 12, 12, 13, 13]
                last_norm = None
                for qb in range(NQB):
                    for h in range(H_CORE):
                        norms = block(h, qb)
                        if (qb, h) == (NQB - 1, H_CORE - 1):
                            last_norm = norms
                        else:
                            for i, nrm in enumerate(norms):
                                aux.append((1 + i, 1, nrm))
                    # out-projection of this q-block: needs every head's
                    # normalize, which lands early in the following block
                    if qb == 0:
                        for j, st in enumerate(range(8)):
                            aux.append((MKT[j], 2, outproj(st, axp)))
                tail_sts = list(range(8, 16))
                leftovers = sorted(aux, key=lambda x: x[0])

            # tail: drain with a deeper pool; interleave the last block's
            # normalize with the out-projections that don't depend on it
            with tc.tile_pool(name="tail_ps", bufs=4, space="PSUM") as tlp:
                for _, _, fn in leftovers:
                    fn()
                last_norm[0]()
                for st in tail_sts[:4]:
                    outproj(st, tlp)()
                last_norm[1]()
                for st in tail_sts[4:]:
                    outproj(st, tlp)()

    nc.compile()
    return nc


def _prep_core(x, wq, bq, wk, bk, wv, bv, wo, bo, b, g):
    hs = slice(DHC * g, DHC * (g + 1))
    xT = np.ascontiguousarray(x[b].T).astype(ml_dtypes.bfloat16)
    wq_c = (wq[:, hs] / 8.0).astype(ml_dtypes.bfloat16)
    bq_c = (bq[hs] / 8.0).astype(np.float32).reshape(DHC, 1)
    wk_c = wk[:, hs].astype(ml_dtypes.bfloat16)
    bk_c = bk[hs].astype(np.float32).reshape(DHC, 1)
    wv_aug = np.zeros((D, VW), np.float32)
    bvb = np.zeros((128, VW), np.float32)
    for h in range(H_CORE):
        c0 = 2 * HD * h
        wv_aug[:, c0:c0 + HD] = wv[:, DHC * g + HD * h:DHC * g + HD * (h + 1)]
        bvb[:, c0:c0 + HD] = bv[DHC * g + HD * h:DHC * g + HD * (h + 1)][None, :]
        bvb[:, c0 + HD:c0 + 2 * HD] = 1.0
    wo_c = wo[hs, :].astype(ml_dtypes.bfloat16)
    return {
        "xT": xT,
        "wq": wq_c, "bq": bq_c,
        "wk": wk_c, "bk": bk_c,
        "wv": wv_aug.astype(ml_dtypes.bfloat16), "bvb": bvb,
        "wo": wo_c,
    }


def kernel(x, wq, bq, wk, bk, wv, bv, wo, bo):
    x = np.asarray(x, np.float32)
    wq, bq = np.asarray(wq, np.float32), np.asarray(bq, np.float32)
    wk, bk = np.asarray(wk, np.float32), np.asarray(bk, np.float32)
    wv, bv = np.asarray(wv, np.float32), np.asarray(bv, np.float32)
    wo, bo = np.asarray(wo, np.float32), np.asarray(bo, np.float32)

    if "nc" not in _CACHE:
        _CACHE["nc"] = build_nc()
    nc = _CACHE["nc"]

    in_maps = []
    for c in range(N_CORES):
        b, g = divmod(c, 2)
        in_maps.append(_prep_core(x, wq, bq, wk, bk, wv, bv, wo, bo, b, g))

    res = run_bass_kernel_spmd(nc, in_maps, list(range(N_CORES)))

    out = np.empty((B, S, D), np.float32)
    for b in range(B):
        out[b] = (res.results[2 * b]["out"] + res.results[2 * b + 1]["out"]
                  + bo[None, :])
    return out
